# revision 9
# baseline (speedup 1.0000x reference)
"""MobileMamba block kernel for 8x Trainium2 NeuronCores — v3.

Math restructure:
  xc   = silu(x @ w1.T + b1)                          # [E, L] channel-major
  c    = depthwise_conv5(xc) (+bd, BN affine folded)
  xl   = silu(c)
  SSM with constant B/C collapses to a scalar first-order recurrence:
    xs = (CB/Dv) * xl      (prescale: Act Identity(scale) or DVE ts 4x)
    g  = a*g + xs          (DVE tensor_tensor_scan, pair-chained)
    gp = g + xl            (tensor_tensor add: Pool tiles 0-1, DVE 2-3)
  out  = w2dv @ gp + b2    (w2dv = w2.T * Dv)

Scheduling insight from HW traces: the PE streams back-to-back 512-col
bf16 matmuls at a 216ns cadence (0.42 ns/col, LDWEIGHTS fully hidden),
so ALL of mm1/conv/mm2 lives on PE and the kernel is bound by the DVE
scan path plus ramp/tail. Everything is sized to keep the PE queue dense
and to start the DVE scan as early as possible.

Pipeline (chunk c = 512, pair p = 1024):
  PE:  mm1 c0,c1,c2 | conv p0 | mm1 c3 | conv p1 | mm2 c0..c3
  Act: silu1 (512 from psA), silu2 (1024 from psB pair), xs for tiles
       0-1, out-copy+bias (512 from psC)
  DVE: per pair, per tile: [xs tiles 2-3], scan, [gp tiles 2-3]
  Pool: gp tiles 0-1, constants staging junk
conv reads a zero-padded xc (2 pad cols per side per tile) so every tap
is a full-width matmul; pair p0's +taps read 2 cols into chunk c2, hence
mm1 c2 precedes conv p0.

Sharding: data-parallel over batch (B=8 -> 8 cores), one sample per core.
"""

import sys

for _p in ('/opt/trn_rl_repo',):
    if _p not in sys.path:
        sys.path.append(_p)

import numpy as np

import concourse.bass as bass
import concourse.tile as tile
from concourse import mybir

D = 256      # model dim
E = 512      # expanded dim
L = 2048     # sequence length
NCORES = 8
BN_EPS = 1e-5

F32 = mybir.dt.float32
BF16 = mybir.dt.bfloat16

EM = E // 128   # 4 channel tiles
DM = D // 128   # 2 model-dim tiles
CH = 512        # chunk
LC = L // CH    # 4 chunks
PR = 1024       # pair
NP = L // PR    # 2 pairs

TW = L + 4      # padded tile width in xc (2 zero cols each side)
TAPS = (-2, -1, 0, 1, 2)

# mpc (f32 per-channel param) columns
PC_B1 = 0        # 4 cols: b1 per tile
PC_CBIAS = 4     # 4 cols: conv+bn bias per tile
PC_CBDV = 8      # 4 cols: CB/Dv per tile
PC_B2 = 12       # 2 cols: b2 per dt
PC_NCOL = 14

ACT_XS_TILES = (0, 1)    # xs prescale on Act
DVE_XS_TILES = (2, 3)    # xs prescale on DVE
POOL_GP_TILES = (0, 1)   # gp add on Pool
DVE_GP_TILES = (2, 3)


def build_nc():
    nc = bass.Bass()
    xt = nc.declare_dram_parameter("xt", [D, L], BF16, isOutput=False)
    mw1 = nc.declare_dram_parameter("mw1", [128, DM * E], BF16, isOutput=False)
    mw2 = nc.declare_dram_parameter("mw2", [128, EM * D], BF16, isOutput=False)
    mdg = nc.declare_dram_parameter("mdg", [128, EM * 5 * 128], BF16,
                                    isOutput=False)
    mae = nc.declare_dram_parameter("mae", [128, EM * PR], BF16, isOutput=False)
    mpc = nc.declare_dram_parameter("mpc", [128, PC_NCOL], F32, isOutput=False)
    outT = nc.declare_dram_parameter("outT", [D, L], F32, isOutput=True)

    with tile.TileContext(nc) as tc:
        with (
            tc.tile_pool(name="const", bufs=1) as const,
            tc.tile_pool(name="acts", bufs=1) as acts,
            tc.tile_pool(name="psA", bufs=2, space="PSUM") as psA,
            tc.tile_pool(name="psB", bufs=2, space="PSUM") as psB,
            tc.tile_pool(name="psC", bufs=2, space="PSUM") as psC,
        ):
            # ---------- DMAs, ordered so x lands just in time ----------
            mpc_t = const.tile([128, PC_NCOL], F32)
            nc.scalar.dma_start(out=mpc_t, in_=mpc[:, :])
            mw1_t = const.tile([128, DM * E], BF16)
            nc.sync.dma_start(out=mw1_t, in_=mw1[:, :])
            # x: one SBUF tile PER CHUNK (per-tile dep tracking: a chunk's
            # consumer must not wait later chunks' DMAs)
            xtc = [const.tile([128, DM * CH], BF16, name=f"xtc{lc}",
                              tag=f"xtc{lc}") for lc in range(LC)]
            for h in range(2):  # chunk 0 striped for latency
                for k in range(DM):
                    nc.sync.dma_start(
                        out=xtc[0][:, k * CH + h * 256:k * CH + (h + 1) * 256],
                        in_=xt[k * 128:(k + 1) * 128, h * 256:(h + 1) * 256])
            for lc in range(1, LC):
                for k in range(DM):
                    nc.sync.dma_start(
                        out=xtc[lc][:, k * CH:(k + 1) * CH],
                        in_=xt[k * 128:(k + 1) * 128, lc * CH:(lc + 1) * CH])
            mdg_t = const.tile([128, EM * 5 * 128], BF16)
            nc.gpsimd.dma_start(out=mdg_t, in_=mdg[:, :])
            mae_t = const.tile([128, EM * PR], BF16)
            nc.gpsimd.dma_start(out=mae_t, in_=mae[:, :])
            mw2_t = const.tile([128, EM * D], BF16)
            nc.gpsimd.dma_start(out=mw2_t, in_=mw2[:, :])

            # ---------- SBUF activations (flat tensors) ----------
            xc = acts.tile([128, EM * TW], BF16, name="xc", tag="xc")
            xl = acts.tile([128, EM * L], BF16, name="xl", tag="xl")
            xs = acts.tile([128, EM * L], BF16, name="xs", tag="xs")
            g = acts.tile([128, EM * L], BF16, name="g", tag="g")
            gp = acts.tile([128, EM * L], BF16, name="gp", tag="gp")
            osb = acts.tile([128, DM * L], F32, name="osb", tag="osb")

            # zero the xc pad columns (2 each side per tile)
            for m in range(EM):
                nc.gpsimd.memset(xc[:, m * TW:m * TW + 2], 0.0)
                nc.gpsimd.memset(xc[:, m * TW + 2 + L:(m + 1) * TW], 0.0)

            # ---------- per-engine touches (collapse DMA sem waits) ----------
            ps_scr = psA.tile([128, 8], F32, name="ps_scr", tag="psA")
            nc.tensor.matmul(out=ps_scr[:, 0:4], lhsT=mw1_t[:, 0:128],
                             rhs=mw1_t[:, 0:4], start=True, stop=True)
            v_scr = const.tile([128, 1], F32)
            nc.vector.tensor_copy(out=v_scr, in_=mpc_t[:, 0:1])
            a_scr = const.tile([128, 1], F32)
            nc.scalar.copy(out=a_scr, in_=mpc_t[:, 0:1])

            # ---------- helpers ----------
            def xc_ap(m, t0, n):
                return xc[:, m * TW + 2 + t0:m * TW + 2 + t0 + n]

            def pcol(c):
                return mpc_t[:, c:c + 1]

            w1s = [mw1_t[:, k * E:(k + 1) * E] for k in range(DM)]

            def mm1_chunk(m, lc):
                ps1 = psA.tile([128, CH], F32, name="ps1", tag="psA")
                for k in range(DM):
                    nc.tensor.matmul(
                        out=ps1,
                        lhsT=w1s[k][:, m * 128:(m + 1) * 128],
                        rhs=xtc[lc][:, k * CH:(k + 1) * CH],
                        start=(k == 0), stop=(k == DM - 1))
                nc.scalar.activation(
                    out=xc_ap(m, lc * CH, CH), in_=ps1,
                    func=mybir.ActivationFunctionType.Silu,
                    bias=pcol(PC_B1 + m), scale=1.0)

            def conv_pair_mm(m, p):
                """10 diag matmuls into a 2-bank PSUM pair."""
                a0 = p * PR
                ps2 = psB.tile([128, PR], F32, name="ps2", tag="psB")
                for j, dlt in enumerate(TAPS):
                    dg = mdg_t[:, (m * 5 + j) * 128:(m * 5 + j + 1) * 128]
                    for h in range(2):
                        nc.tensor.matmul(
                            out=ps2[:, h * CH:(h + 1) * CH],
                            lhsT=dg, rhs=xc_ap(m, a0 + h * CH + dlt, CH),
                            start=(j == 0), stop=(j == len(TAPS) - 1))
                return ps2

            def silu2_pair(m, p, ps2):
                a0 = m * L + p * PR
                nc.scalar.activation(
                    out=xl[:, a0:a0 + PR], in_=ps2,
                    func=mybir.ActivationFunctionType.Silu,
                    bias=pcol(PC_CBIAS + m), scale=1.0)

            def xs_pair(m, p):
                a0 = m * L + p * PR
                nc.vector.tensor_scalar(
                    out=xs[:, a0:a0 + PR], in0=xl[:, a0:a0 + PR],
                    scalar1=pcol(PC_CBDV + m), scalar2=None,
                    op0=mybir.AluOpType.mult)

            def scan_pair(m, p):
                a0 = m * L + p * PR
                nc.vector.tensor_tensor_scan(
                    out=g[:, a0:a0 + PR], data0=mae_t[:, m * PR:(m + 1) * PR],
                    data1=xs[:, a0:a0 + PR],
                    initial=(0.0 if p == 0 else g[:, a0 - 1:a0]),
                    op0=mybir.AluOpType.mult, op1=mybir.AluOpType.add)

            def gp_pair(m, p):
                a0 = m * L + p * PR
                eng = nc.gpsimd if m in POOL_GP_TILES else nc.vector
                eng.tensor_tensor(
                    out=gp[:, a0:a0 + PR], in0=g[:, a0:a0 + PR],
                    in1=xl[:, a0:a0 + PR], op=mybir.AluOpType.add)

            def mm2_start(dt, lc, ecs):
                a0 = lc * CH
                ps3 = psC.tile([128, CH], F32, name="ps3", tag="psC")
                for ec in ecs:
                    nc.tensor.matmul(
                        out=ps3,
                        lhsT=mw2_t[:, ec * D + dt * 128:ec * D + (dt + 1) * 128],
                        rhs=gp[:, ec * L + a0:ec * L + a0 + CH],
                        start=(ec == 0), stop=(ec == EM - 1),
                        skip_group_check=True)
                return ps3

            def mm2_finish(dt, lc, ps3, ecs):
                a0 = lc * CH
                for ec in ecs:
                    nc.tensor.matmul(
                        out=ps3,
                        lhsT=mw2_t[:, ec * D + dt * 128:ec * D + (dt + 1) * 128],
                        rhs=gp[:, ec * L + a0:ec * L + a0 + CH],
                        start=(ec == 0), stop=(ec == EM - 1),
                        skip_group_check=True)
                nc.scalar.activation(
                    out=osb[:, dt * L + a0:dt * L + a0 + CH], in_=ps3,
                    func=mybir.ActivationFunctionType.Identity,
                    bias=pcol(PC_B2 + dt), scale=1.0)
                nc.gpsimd.dma_start(
                    out=outT[dt * 128:(dt + 1) * 128, a0:a0 + CH],
                    in_=osb[:, dt * L + a0:dt * L + a0 + CH])

            def mm2_chunk(dt, lc):
                ps3 = mm2_start(dt, lc, range(EM - 1))
                mm2_finish(dt, lc, ps3, (EM - 1,))

            # ---------- emission ----------
            for lc in (0, 1, 2):
                for m in range(EM):
                    mm1_chunk(m, lc)

            def scan_block(p):
                for m in range(EM):
                    xs_pair(m, p)
                    scan_pair(m, p)
                    gp_pair(m, p)

            # conv pair 0 (PE) + per-tile silu2/scan path; mm1 c3 wedged in
            # after tile-1's conv so silu1 c3 is ready before conv p1
            for m in range(EM):
                ps2 = conv_pair_mm(m, 0)
                silu2_pair(m, 0, ps2)
                xs_pair(m, 0)
                scan_pair(m, 0)
                gp_pair(m, 0)
                if m == 1:
                    for mm in range(EM):
                        mm1_chunk(mm, 3)
            for m in range(EM):
                ps2 = conv_pair_mm(m, 1)
                silu2_pair(m, 1, ps2)
                xs_pair(m, 1)
                scan_pair(m, 1)
                gp_pair(m, 1)
            for lc in (0, 1):
                for dt in range(DM):
                    mm2_chunk(dt, lc)
            for dt in range(DM):
                mm2_chunk(dt, 2)
            # tail: run ec 0-2 of chunk 3 first; the ec-3 matmuls (gp of
            # tile 3, the last scan to land) go last
            tails = [(dt, 3, mm2_start(dt, 3, range(EM - 1)))
                     for dt in range(DM)]
            for dt, lc, ps3 in tails:
                mm2_finish(dt, lc, ps3, (EM - 1,))

    _split_waits(nc)
    return nc


_WSPLIT_SKIP = ("InstAllEngineBarrier", "InstNoOp",
                "InstEventSemaphore", "InstUnconditionalBranch")


def _split_waits(nc, max_waits=1):
    """Walrus allows one sync-wait command per TPB instruction; spill extra
    waits onto same-engine NoOps."""
    n_split = 0
    for f in nc.m.functions:
        for bb in f.blocks:
            out = []
            for inst in bb.instructions:
                si = inst.sync_info
                waits = list(si.on_wait) if si and si.on_wait else []
                if (len(waits) > max_waits
                        and inst.__class__.__name__ not in _WSPLIT_SKIP):
                    spill, keep = waits[:-max_waits], waits[-max_waits:]
                    for i, w in enumerate(spill):
                        out.append(mybir.InstNoOp(
                            name=f"{inst.name}_ws{i}",
                            engine=inst.engine,
                            sync_info=mybir.SyncInfo(on_wait=[w],
                                                     on_update=[]),
                        ))
                        n_split += 1
                    si.on_wait = keep
                out.append(inst)
            if n_split:
                bb.instructions = out
    return nc


def _to_bf16(a):
    import ml_dtypes
    return np.asarray(a, np.float32).astype(ml_dtypes.bfloat16)


def host_params(w1, b1, wd, bd, gamma, beta, rmean, rvar, A, Bm, Cm, Dv, w2, b2):
    s = (gamma / np.sqrt(rvar + BN_EPS)).astype(np.float32)
    cw = (wd[:, 0, :] * s[:, None]).astype(np.float32)            # [E, 5]
    cbias = (bd * s + beta - rmean * s).astype(np.float32)        # [E]
    expA = np.exp(np.asarray(A, np.float32))                      # [E]
    CB = (np.asarray(Bm, np.float32) * np.asarray(Cm, np.float32)).sum(1)
    w1t = np.asarray(w1, np.float32).T                            # [D, E]
    w2t = np.asarray(w2, np.float32).T                            # [E, D]

    dv = np.asarray(Dv, np.float32).copy()
    tiny = np.abs(dv) < 1e-6
    dv[tiny] = np.where(dv[tiny] < 0, -1e-6, 1e-6)
    cbdv = (CB / dv).astype(np.float32)

    mw1 = np.zeros((128, DM * E), np.float32)
    for k in range(DM):
        mw1[:, k * E:(k + 1) * E] = w1t[k * 128:(k + 1) * 128, :]

    mw2 = np.zeros((128, EM * D), np.float32)
    for ec in range(EM):
        mw2[:, ec * D:(ec + 1) * D] = \
            w2t[ec * 128:(ec + 1) * 128, :] * dv[ec * 128:(ec + 1) * 128, None]

    mdg = np.zeros((128, EM * 5 * 128), np.float32)
    for m in range(EM):
        for j in range(5):
            blk = np.zeros((128, 128), np.float32)
            np.fill_diagonal(blk, cw[m * 128:(m + 1) * 128, j])
            mdg[:, (m * 5 + j) * 128:(m * 5 + j + 1) * 128] = blk

    mae = np.zeros((128, EM * PR), np.float32)
    for m in range(EM):
        mae[:, m * PR:(m + 1) * PR] = expA[m * 128:(m + 1) * 128, None]

    mpc = np.zeros((128, PC_NCOL), np.float32)
    for m in range(EM):
        sl = slice(m * 128, (m + 1) * 128)
        mpc[:, PC_B1 + m] = np.asarray(b1, np.float32)[sl]
        mpc[:, PC_CBIAS + m] = cbias[sl]
        mpc[:, PC_CBDV + m] = cbdv[sl]
    for dt in range(DM):
        mpc[:, PC_B2 + dt] = np.asarray(b2, np.float32)[dt * 128:(dt + 1) * 128]

    return dict(mw1=_to_bf16(mw1), mw2=_to_bf16(mw2), mdg=_to_bf16(mdg),
                mae=_to_bf16(mae), mpc=mpc)


_CACHED_NC = None


def kernel(x, w1, b1, wd, bd, gamma, beta, rmean, rvar, A, Bm, Cm, Dv, w2, b2,
           **run_kwargs):
    from concourse.bass_utils import run_bass_kernel_spmd
    global _CACHED_NC
    if _CACHED_NC is None:
        _CACHED_NC = build_nc()
    nc = _CACHED_NC

    params = host_params(w1, b1, wd, bd, gamma, beta, rmean, rvar,
                         A, Bm, Cm, Dv, w2, b2)
    x = np.asarray(x, dtype=np.float32)
    in_maps = []
    for i in range(NCORES):
        m = dict(params)
        m["xt"] = _to_bf16(np.ascontiguousarray(x[i].T))  # [D, L] bf16
        in_maps.append(m)

    res = run_bass_kernel_spmd(nc, in_maps, core_ids=list(range(NCORES)),
                               **run_kwargs)
    out = np.stack([np.asarray(r["outT"]).T for r in res.results])  # [B, L, D]
    if run_kwargs:
        kernel.last_result = res
    return out


# revision 14
# speedup vs baseline: 1.2104x; 1.2104x over previous
"""MobileMamba block kernel for 8x Trainium2 NeuronCores — v3.

Math restructure:
  xc   = silu(x @ w1.T + b1)                          # [E, L] channel-major
  c    = depthwise_conv5(xc) (+bd, BN affine folded)
  xl   = silu(c)
  SSM with constant B/C collapses to a scalar first-order recurrence:
    xs = (CB/Dv) * xl      (prescale: Act Identity(scale) or DVE ts 4x)
    g  = a*g + xs          (DVE tensor_tensor_scan, pair-chained)
    gp = g + xl            (tensor_tensor add: Pool tiles 0-1, DVE 2-3)
  out  = w2dv @ gp + b2    (w2dv = w2.T * Dv)

Scheduling insight from HW traces: the PE streams back-to-back 512-col
bf16 matmuls at a 216ns cadence (0.42 ns/col, LDWEIGHTS fully hidden),
so ALL of mm1/conv/mm2 lives on PE and the kernel is bound by the DVE
scan path plus ramp/tail. Everything is sized to keep the PE queue dense
and to start the DVE scan as early as possible.

Pipeline (chunk c = 512, pair p = 1024):
  PE:  mm1 c0,c1,c2 | conv p0 | mm1 c3 | conv p1 | mm2 c0..c3
  Act: silu1 (512 from psA), silu2 (1024 from psB pair), xs for tiles
       0-1, out-copy+bias (512 from psC)
  DVE: per pair, per tile: [xs tiles 2-3], scan, [gp tiles 2-3]
  Pool: gp tiles 0-1, constants staging junk
conv reads a zero-padded xc (2 pad cols per side per tile) so every tap
is a full-width matmul; pair p0's +taps read 2 cols into chunk c2, hence
mm1 c2 precedes conv p0.

Sharding: data-parallel over batch (B=8 -> 8 cores), one sample per core.
"""

import sys

for _p in ('/opt/trn_rl_repo',):
    if _p not in sys.path:
        sys.path.append(_p)

import numpy as np

import concourse.bass as bass
import concourse.tile as tile
from concourse import mybir

D = 256      # model dim
E = 512      # expanded dim
L = 2048     # sequence length
NCORES = 8
BN_EPS = 1e-5

F32 = mybir.dt.float32
BF16 = mybir.dt.bfloat16

EM = E // 128   # 4 channel tiles
DM = D // 128   # 2 model-dim tiles
CH = 512        # chunk
LC = L // CH    # 4 chunks
PR = 1024       # pair
NP = L // PR    # 2 pairs

TW = L + 4      # padded tile width in xc (2 zero cols each side)
TAPS = (-2, -1, 0, 1, 2)

# mpc (f32 per-channel param) columns
PC_B1 = 0        # 4 cols: b1 per tile
PC_CBIAS = 4     # 4 cols: conv+bn bias per tile
PC_CBDV = 8      # 4 cols: CB/Dv per tile
PC_B2 = 12       # 2 cols: b2 per dt
PC_NCOL = 14

ACT_XS_TILES = (0, 1)    # xs prescale on Act
DVE_XS_TILES = (2, 3)    # xs prescale on DVE
POOL_GP_TILES = (0, 1)   # gp add on Pool
DVE_GP_TILES = (2, 3)


def build_nc():
    nc = bass.Bass()
    xt = nc.declare_dram_parameter("xt", [D, L], BF16, isOutput=False)
    mw1 = nc.declare_dram_parameter("mw1", [128, DM * E], BF16, isOutput=False)
    mw2 = nc.declare_dram_parameter("mw2", [128, EM * D], BF16, isOutput=False)
    mdg = nc.declare_dram_parameter("mdg", [128, EM * 5 * 128], BF16,
                                    isOutput=False)
    mae = nc.declare_dram_parameter("mae", [128, EM * PR], BF16, isOutput=False)
    mpc = nc.declare_dram_parameter("mpc", [128, PC_NCOL], F32, isOutput=False)
    outT = nc.declare_dram_parameter("outT", [D, L], F32, isOutput=True)

    with tile.TileContext(nc) as tc:
        with (
            tc.tile_pool(name="const", bufs=1) as const,
            tc.tile_pool(name="acts", bufs=1) as acts,
            tc.tile_pool(name="psA", bufs=2, space="PSUM") as psA,
            tc.tile_pool(name="psB", bufs=2, space="PSUM") as psB,
            tc.tile_pool(name="psC", bufs=2, space="PSUM") as psC,
        ):
            # ---------- input staging ----------
            # Each dma_start runs on one HW ring (~23B/ns); separate DMAs
            # parallelize across rings but cost ~0.6us issue time on the
            # issuing engine's queue. Stripe inputs and spread issues over
            # all four DGEs; separate SBUF tiles per consumer group so the
            # per-tile dep tracking never makes an early consumer wait on a
            # late DMA.
            xtc = [const.tile([128, DM * CH], BF16, name=f"xtc{lc}",
                              tag=f"xtc{lc}") for lc in range(LC)]
            mw1A = const.tile([128, 2 * DM * 128], BF16)  # m0,m1 (m-major)
            mw1B = const.tile([128, 2 * DM * 128], BF16)  # m2,m3
            mdgt = [const.tile([128, 5 * 128], BF16, name=f"mdg{m}",
                               tag=f"mdg{m}") for m in range(EM)]
            mae_t = const.tile([128, EM * PR], BF16)
            mw2_t = const.tile([128, EM * D], BF16)
            mpc_t = const.tile([128, PC_NCOL], F32)

            # SP: x chunks 0-2 in 64KB stripes, chunk 3 in two halves
            for lc in (0, 1, 2):
                for k in range(DM):
                    for h in range(2):
                        c0, w = lc * CH + h * 256, 256
                        nc.sync.dma_start(
                            out=xtc[lc][:, k * CH + h * 256:k * CH + h * 256 + w],
                            in_=xt[k * 128:(k + 1) * 128, c0:c0 + w])
            for k in range(DM):
                nc.sync.dma_start(
                    out=xtc[3][:, k * CH:(k + 1) * CH],
                    in_=xt[k * 128:(k + 1) * 128, 3 * CH:4 * CH])
            # Act: mpc then per-tile diag matrices
            nc.scalar.dma_start(out=mpc_t, in_=mpc[:, :])
            for m in range(EM):
                nc.scalar.dma_start(out=mdgt[m],
                                    in_=mdg[:, m * 640:(m + 1) * 640])
            # Pool: w1 (m-major 32KB stripes), mae, w2
            for m in range(EM):
                dst = mw1A if m < 2 else mw1B
                for k in range(DM):
                    c0 = ((m % 2) * DM + k) * 128
                    nc.gpsimd.dma_start(out=dst[:, c0:c0 + 128],
                                        in_=mw1[:, (k * EM + m) * 128:
                                                   (k * EM + m + 1) * 128])
                # NOTE: host packs mw1 m-major per k: col (k*EM+m)*128
            for h in range(2):
                nc.gpsimd.dma_start(
                    out=mae_t[:, h * 2 * PR:(h + 1) * 2 * PR],
                    in_=mae[:, h * 2 * PR:(h + 1) * 2 * PR])
            for h in range(2):
                nc.gpsimd.dma_start(
                    out=mw2_t[:, h * EM * 128:(h + 1) * EM * 128],
                    in_=mw2[:, h * EM * 128:(h + 1) * EM * 128])

            # ---------- SBUF activations (flat tensors) ----------
            xc = acts.tile([128, EM * TW], BF16, name="xc", tag="xc")
            xl = acts.tile([128, EM * L], BF16, name="xl", tag="xl")
            xs = acts.tile([128, EM * L], BF16, name="xs", tag="xs")
            g = acts.tile([128, EM * L], BF16, name="g", tag="g")
            gp = acts.tile([128, EM * L], BF16, name="gp", tag="gp")
            osb = acts.tile([128, DM * L], F32, name="osb", tag="osb")

            # zero the xc pad columns (2 each side per tile)
            for m in range(EM):
                nc.gpsimd.memset(xc[:, m * TW:m * TW + 2], 0.0)
                nc.gpsimd.memset(xc[:, m * TW + 2 + L:(m + 1) * TW], 0.0)

            # ---------- per-engine touches (collapse DMA sem waits) ----------
            ps_scr = psA.tile([128, 8], F32, name="ps_scr", tag="psA")
            nc.tensor.matmul(out=ps_scr[:, 0:4], lhsT=mw1A[:, 0:128],
                             rhs=mw1A[:, 0:4], start=True, stop=True)
            v_scr = const.tile([128, 1], F32)
            nc.vector.tensor_copy(out=v_scr, in_=mpc_t[:, 0:1])
            a_scr = const.tile([128, 1], F32)
            nc.scalar.copy(out=a_scr, in_=mpc_t[:, 0:1])

            # ---------- helpers ----------
            def xc_ap(m, t0, n):
                return xc[:, m * TW + 2 + t0:m * TW + 2 + t0 + n]

            def pcol(c):
                return mpc_t[:, c:c + 1]

            def w1_ap(m, k):
                t = mw1A if m < 2 else mw1B
                c0 = ((m % 2) * DM + k) * 128
                return t[:, c0:c0 + 128]

            def mm1_chunk(m, lc):
                ps1 = psA.tile([128, CH], F32, name="ps1", tag="psA")
                for k in range(DM):
                    nc.tensor.matmul(
                        out=ps1,
                        lhsT=w1_ap(m, k),
                        rhs=xtc[lc][:, k * CH:(k + 1) * CH],
                        start=(k == 0), stop=(k == DM - 1))
                nc.scalar.activation(
                    out=xc_ap(m, lc * CH, CH), in_=ps1,
                    func=mybir.ActivationFunctionType.Silu,
                    bias=pcol(PC_B1 + m), scale=1.0)

            def conv_pair_mm(m, p):
                """2x5 diag matmuls into a 2-bank PSUM pair (h-outer so the
                two accumulation groups stay sequential)."""
                a0 = p * PR
                ps2 = psB.tile([128, PR], F32, name="ps2", tag="psB")
                for h in range(2):
                    for j, dlt in enumerate(TAPS):
                        dg = mdgt[m][:, j * 128:(j + 1) * 128]
                        nc.tensor.matmul(
                            out=ps2[:, h * CH:(h + 1) * CH],
                            lhsT=dg, rhs=xc_ap(m, a0 + h * CH + dlt, CH),
                            start=(j == 0), stop=(j == len(TAPS) - 1))
                return ps2

            def silu2_pair(m, p, ps2):
                a0 = m * L + p * PR
                nc.scalar.activation(
                    out=xl[:, a0:a0 + PR], in_=ps2,
                    func=mybir.ActivationFunctionType.Silu,
                    bias=pcol(PC_CBIAS + m), scale=1.0)

            def xs_pair(m, p):
                a0 = m * L + p * PR
                nc.vector.tensor_scalar(
                    out=xs[:, a0:a0 + PR], in0=xl[:, a0:a0 + PR],
                    scalar1=pcol(PC_CBDV + m), scalar2=None,
                    op0=mybir.AluOpType.mult)

            def scan_pair(m, p):
                a0 = m * L + p * PR
                nc.vector.tensor_tensor_scan(
                    out=g[:, a0:a0 + PR], data0=mae_t[:, m * PR:(m + 1) * PR],
                    data1=xs[:, a0:a0 + PR],
                    initial=(0.0 if p == 0 else g[:, a0 - 1:a0]),
                    op0=mybir.AluOpType.mult, op1=mybir.AluOpType.add)

            def gp_pair(m, p):
                a0 = m * L + p * PR
                eng = nc.gpsimd if m in POOL_GP_TILES else nc.vector
                eng.tensor_tensor(
                    out=gp[:, a0:a0 + PR], in0=g[:, a0:a0 + PR],
                    in1=xl[:, a0:a0 + PR], op=mybir.AluOpType.add)

            def mm2_start(dt, lc, ecs):
                a0 = lc * CH
                ps3 = psC.tile([128, CH], F32, name="ps3", tag="psC")
                for ec in ecs:
                    nc.tensor.matmul(
                        out=ps3,
                        lhsT=mw2_t[:, ec * D + dt * 128:ec * D + (dt + 1) * 128],
                        rhs=gp[:, ec * L + a0:ec * L + a0 + CH],
                        start=(ec == 0), stop=(ec == EM - 1),
                        skip_group_check=True)
                return ps3

            def mm2_finish(dt, lc, ps3, ecs):
                a0 = lc * CH
                for ec in ecs:
                    nc.tensor.matmul(
                        out=ps3,
                        lhsT=mw2_t[:, ec * D + dt * 128:ec * D + (dt + 1) * 128],
                        rhs=gp[:, ec * L + a0:ec * L + a0 + CH],
                        start=(ec == 0), stop=(ec == EM - 1),
                        skip_group_check=True)
                nc.scalar.activation(
                    out=osb[:, dt * L + a0:dt * L + a0 + CH], in_=ps3,
                    func=mybir.ActivationFunctionType.Identity,
                    bias=pcol(PC_B2 + dt), scale=1.0)
                nc.gpsimd.dma_start(
                    out=outT[dt * 128:(dt + 1) * 128, a0:a0 + CH],
                    in_=osb[:, dt * L + a0:dt * L + a0 + CH])

            def mm2_chunk(dt, lc):
                ps3 = mm2_start(dt, lc, range(EM - 1))
                mm2_finish(dt, lc, ps3, (EM - 1,))

            # ---------- emission ----------
            for lc in (0, 1, 2):
                for m in range(EM):
                    mm1_chunk(m, lc)

            def scan_block(p):
                for m in range(EM):
                    xs_pair(m, p)
                    scan_pair(m, p)
                    gp_pair(m, p)

            # conv pair 0 (PE) + per-tile silu2/scan path
            for m in range(EM):
                ps2 = conv_pair_mm(m, 0)
                silu2_pair(m, 0, ps2)
                xs_pair(m, 0)
                scan_pair(m, 0)
                gp_pair(m, 0)
            for mm in range(EM):
                mm1_chunk(mm, 3)
            for m in range(EM):
                ps2 = conv_pair_mm(m, 1)
                silu2_pair(m, 1, ps2)
                xs_pair(m, 1)
                scan_pair(m, 1)
                gp_pair(m, 1)
            for lc in (0, 1):
                for dt in range(DM):
                    mm2_chunk(dt, lc)
            for dt in range(DM):
                mm2_chunk(dt, 2)
            # tail: run ec 0-2 of chunk 3 first; the ec-3 matmuls (gp of
            # tile 3, the last scan to land) go last
            tails = [(dt, 3, mm2_start(dt, 3, range(EM - 1)))
                     for dt in range(DM)]
            for dt, lc, ps3 in tails:
                mm2_finish(dt, lc, ps3, (EM - 1,))

    _split_waits(nc)
    return nc


_WSPLIT_SKIP = ("InstAllEngineBarrier", "InstNoOp",
                "InstEventSemaphore", "InstUnconditionalBranch")


def _split_waits(nc, max_waits=1):
    """Walrus allows one sync-wait command per TPB instruction; spill extra
    waits onto same-engine NoOps."""
    n_split = 0
    for f in nc.m.functions:
        for bb in f.blocks:
            out = []
            for inst in bb.instructions:
                si = inst.sync_info
                waits = list(si.on_wait) if si and si.on_wait else []
                if (len(waits) > max_waits
                        and inst.__class__.__name__ not in _WSPLIT_SKIP):
                    spill, keep = waits[:-max_waits], waits[-max_waits:]
                    for i, w in enumerate(spill):
                        out.append(mybir.InstNoOp(
                            name=f"{inst.name}_ws{i}",
                            engine=inst.engine,
                            sync_info=mybir.SyncInfo(on_wait=[w],
                                                     on_update=[]),
                        ))
                        n_split += 1
                    si.on_wait = keep
                out.append(inst)
            if n_split:
                bb.instructions = out
    return nc


def _to_bf16(a):
    import ml_dtypes
    return np.asarray(a, np.float32).astype(ml_dtypes.bfloat16)


def host_params(w1, b1, wd, bd, gamma, beta, rmean, rvar, A, Bm, Cm, Dv, w2, b2):
    s = (gamma / np.sqrt(rvar + BN_EPS)).astype(np.float32)
    cw = (wd[:, 0, :] * s[:, None]).astype(np.float32)            # [E, 5]
    cbias = (bd * s + beta - rmean * s).astype(np.float32)        # [E]
    expA = np.exp(np.asarray(A, np.float32))                      # [E]
    CB = (np.asarray(Bm, np.float32) * np.asarray(Cm, np.float32)).sum(1)
    w1t = np.asarray(w1, np.float32).T                            # [D, E]
    w2t = np.asarray(w2, np.float32).T                            # [E, D]

    dv = np.asarray(Dv, np.float32).copy()
    tiny = np.abs(dv) < 1e-6
    dv[tiny] = np.where(dv[tiny] < 0, -1e-6, 1e-6)
    cbdv = (CB / dv).astype(np.float32)

    mw1 = np.zeros((128, DM * E), np.float32)
    for k in range(DM):
        mw1[:, k * E:(k + 1) * E] = w1t[k * 128:(k + 1) * 128, :]

    mw2 = np.zeros((128, EM * D), np.float32)
    for ec in range(EM):
        mw2[:, ec * D:(ec + 1) * D] = \
            w2t[ec * 128:(ec + 1) * 128, :] * dv[ec * 128:(ec + 1) * 128, None]

    mdg = np.zeros((128, EM * 5 * 128), np.float32)
    for m in range(EM):
        for j in range(5):
            blk = np.zeros((128, 128), np.float32)
            np.fill_diagonal(blk, cw[m * 128:(m + 1) * 128, j])
            mdg[:, (m * 5 + j) * 128:(m * 5 + j + 1) * 128] = blk

    mae = np.zeros((128, EM * PR), np.float32)
    for m in range(EM):
        mae[:, m * PR:(m + 1) * PR] = expA[m * 128:(m + 1) * 128, None]

    mpc = np.zeros((128, PC_NCOL), np.float32)
    for m in range(EM):
        sl = slice(m * 128, (m + 1) * 128)
        mpc[:, PC_B1 + m] = np.asarray(b1, np.float32)[sl]
        mpc[:, PC_CBIAS + m] = cbias[sl]
        mpc[:, PC_CBDV + m] = cbdv[sl]
    for dt in range(DM):
        mpc[:, PC_B2 + dt] = np.asarray(b2, np.float32)[dt * 128:(dt + 1) * 128]

    return dict(mw1=_to_bf16(mw1), mw2=_to_bf16(mw2), mdg=_to_bf16(mdg),
                mae=_to_bf16(mae), mpc=mpc)


_CACHED_NC = None


def kernel(x, w1, b1, wd, bd, gamma, beta, rmean, rvar, A, Bm, Cm, Dv, w2, b2,
           **run_kwargs):
    from concourse.bass_utils import run_bass_kernel_spmd
    global _CACHED_NC
    if _CACHED_NC is None:
        _CACHED_NC = build_nc()
    nc = _CACHED_NC

    params = host_params(w1, b1, wd, bd, gamma, beta, rmean, rvar,
                         A, Bm, Cm, Dv, w2, b2)
    x = np.asarray(x, dtype=np.float32)
    in_maps = []
    for i in range(NCORES):
        m = dict(params)
        m["xt"] = _to_bf16(np.ascontiguousarray(x[i].T))  # [D, L] bf16
        in_maps.append(m)

    res = run_bass_kernel_spmd(nc, in_maps, core_ids=list(range(NCORES)),
                               **run_kwargs)
    out = np.stack([np.asarray(r["outT"]).T for r in res.results])  # [B, L, D]
    if run_kwargs:
        kernel.last_result = res
    return out
